# revision 12
# baseline (speedup 1.0000x reference)
"""HAGMoE kernel for 8 Trainium2 NeuronCores.

Strategy (expert-parallel, sparse dispatch):
  - Host: run the tiny meta/macro routers (numpy fp32), compute the aux
    load-balancing loss, and dispatch tokens by group id (all-to-all done
    host-side since we hold the full inputs anyway).
  - Device: 24 experts sharded 3-per-core. Each (expert, group) task runs
    the residual MLP for its group's tokens (padded to a uniform tile T) in
    a transposed activation layout [feature, token], so every matmul takes
    the HBM weight matrix directly as the stationary lhsT operand and
    streams tokens as the moving operand. fp32 weights are fed to the PE
    as float32r (full-rate fp32 mode, exact enough at free-dim >= 256).
  - Host: sum the 4 micro-expert outputs per group, scale by 1/4, add the
    group-mean fc3 bias, and scatter back to token order.
"""

import os

import numpy as np

META, MACRO, MICRO = 2, 3, 4
D, H, O = 512, 1024, 512
B = 1024
E = META * MACRO * MICRO  # 24
G = META * MACRO          # 6 groups
ALPHA = 0.01
P = 128
NCORES = 8
EPC = E // NCORES         # experts per core = 3

KD = D // P               # 4  k-tiles over D
KH = H // P               # 8  k-tiles over H
MO = O // P               # 4  m-tiles over O

_NC_CACHE = {}
LAST_RESULTS = None
LAST_EXEC_NS = None


def _build_nc(T):
    import concourse.bass as bass
    import concourse.tile as tile
    from concourse import bacc, mybir

    f32 = mybir.dt.float32
    f32r = mybir.dt.float32r
    Relu = mybir.ActivationFunctionType.Relu

    nc = bacc.Bacc("TRN2", debug=False, num_devices=NCORES)

    xts, w1s, w2s, w3s, wps, b1s, outs = [], [], [], [], [], [], []
    for t in range(EPC):
        xts.append(nc.dram_tensor(f"xt{t}", [D, T], f32r, kind="ExternalInput").ap())
        w1s.append(nc.dram_tensor(f"w1_{t}", [D, H], f32r, kind="ExternalInput").ap())
        w2s.append(nc.dram_tensor(f"w2_{t}", [H, H], f32r, kind="ExternalInput").ap())
        w3s.append(nc.dram_tensor(f"w3_{t}", [H, O], f32r, kind="ExternalInput").ap())
        wps.append(nc.dram_tensor(f"wp_{t}", [D, H], f32r, kind="ExternalInput").ap())
        b1s.append(nc.dram_tensor(f"bb_{t}", [P, 2 * KH], f32, kind="ExternalInput").ap())
        outs.append(nc.dram_tensor(f"out{t}", [O, T], f32, kind="ExternalOutput").ap())

    bf16 = mybir.dt.bfloat16

    def absorb(tile_slice):
        # FP32r self-loading matmuls can carry only ONE sync wait in the
        # S3_LW encoding. A dummy bf16 ldweights (garbage load, immediately
        # replaced by the next matmul's self-load) makes the PE observe a
        # fresh DMA semaphore tick so real matmuls never need two waits.
        nc.tensor.ldweights(tile_slice.bitcast(bf16)[:, :128])

    with tile.TileContext(nc) as tc:
        with (
            tc.tile_pool(name="xt", bufs=2) as xt_pool,
            tc.tile_pool(name="w1", bufs=4) as w1_pool,
            tc.tile_pool(name="w2", bufs=2) as w2_pool,
            tc.tile_pool(name="w3", bufs=4) as w3_pool,
            tc.tile_pool(name="wp", bufs=2) as wp_pool,
            tc.tile_pool(name="bias", bufs=2) as b_pool,
            tc.tile_pool(name="babs", bufs=2) as babs_pool,
            tc.tile_pool(name="h1", bufs=2) as h1_pool,
            tc.tile_pool(name="h", bufs=2) as h_pool,
            tc.tile_pool(name="osb", bufs=2) as o_pool,
            tc.tile_pool(name="ps", bufs=8, space="PSUM") as ps_pool,
        ):
            for t in range(EPC):
                # DMA issue order mirrors compute order; split across the
                # two HWDGE rings (SP + ACT) to overlap descriptor work.
                xt = xt_pool.tile([P, KD, T], f32r)
                nc.scalar.dma_start(xt[:], xts[t].rearrange("(ko p) n -> p ko n", p=P))
                bb = b_pool.tile([P, 2 * KH], f32)
                nc.scalar.dma_start(bb[:], b1s[t])
                babs = babs_pool.tile([P, 1], f32)
                nc.scalar.copy(babs[:], bb[:, 0:1])  # absorb bias DMA tick on ACT
                w1h = []
                for ch in range(2):
                    w1c = w1_pool.tile([P, KD, H // 2], f32r, name=f"w1c{t}_{ch}", tag="w1c")
                    nc.sync.dma_start(
                        w1c[:],
                        w1s[t][:, ch * (H // 2):(ch + 1) * (H // 2)].rearrange(
                            "(ko p) n -> p ko n", p=P
                        ),
                    )
                    w1h.append(w1c)

                h1 = h1_pool.tile([P, KH, T], f32r)
                h = h_pool.tile([P, KH, T], f32r)
                osb = o_pool.tile([P, MO, T], f32)

                absorb(xt[:, 0, 0:64])
                absorb(w1h[0][:, 0, 0:64])
                absorb(w1h[1][:, 0, 0:64])

                # stage 1: h1 = relu(W1.T x + b1), layout [H, T]
                for m in range(KH):
                    ps = ps_pool.tile([P, T], f32, name=f"ps1_{t}_{m}", tag="ps")
                    w1c = w1h[m // 4]
                    mm = m % 4
                    for k in range(KD):
                        nc.tensor.matmul(
                            ps,
                            lhsT=w1c[:, k, mm * P:(mm + 1) * P],
                            rhs=xt[:, k, :],
                            start=(k == 0),
                            stop=(k == KD - 1),
                        )
                    nc.scalar.activation(h1[:, m, :], ps, Relu, bias=bb[:, m:m + 1])

                # stage 2: h = relu(W2.T h1 + Wp.T x + (b2+bp))
                # W2 streamed in two half-K chunks; 8 PSUM banks live at once.
                pss = [
                    ps_pool.tile([P, T], f32, name=f"pss{t}_{m}", tag="ps")
                    for m in range(KH)
                ]
                w2c = []
                for kc in range(2):
                    c = w2_pool.tile([P, KD, H], f32r)
                    nc.sync.dma_start(
                        c[:],
                        w2s[t][kc * (H // 2):(kc + 1) * (H // 2), :].rearrange(
                            "(ko p) n -> p ko n", p=P
                        ),
                    )
                    w2c.append(c)
                wp = wp_pool.tile([P, KD, H], f32r)
                nc.scalar.dma_start(wp[:], wps[t].rearrange("(ko p) n -> p ko n", p=P))
                for kc in range(2):
                    absorb(w2c[kc][:, 0, 0:64])
                absorb(wp[:, 0, 0:64])
                for kc in range(2):
                    for m in range(KH):
                        for k in range(KD):
                            nc.tensor.matmul(
                                pss[m],
                                lhsT=w2c[kc][:, k, m * P:(m + 1) * P],
                                rhs=h1[:, kc * KD + k, :],
                                start=(kc == 0 and k == 0),
                                stop=False,
                            )
                for m in range(KH):
                    for k in range(KD):
                        nc.tensor.matmul(
                            pss[m],
                            lhsT=wp[:, k, m * P:(m + 1) * P],
                            rhs=xt[:, k, :],
                            start=False,
                            stop=(k == KD - 1),
                        )
                    nc.scalar.activation(h[:, m, :], pss[m], Relu, bias=bb[:, KH + m:KH + m + 1])

                # stage 3: out = W3.T h, layout [O, T]; W3 in column halves so
                # the tail dependency after the final DMA byte is tiny.
                w3h = []
                for ch in range(2):
                    w3c = w3_pool.tile([P, KH, O // 2], f32r, name=f"w3c{t}_{ch}", tag="w3c")
                    nc.scalar.dma_start(
                        w3c[:],
                        w3s[t][:, ch * (O // 2):(ch + 1) * (O // 2)].rearrange(
                            "(ko p) n -> p ko n", p=P
                        ),
                    )
                    w3h.append(w3c)
                    absorb(w3c[:, 0, 0:64])
                for m in range(MO):
                    ps = ps_pool.tile([P, T], f32, name=f"ps3_{t}_{m}", tag="ps")
                    w3c = w3h[m // 2]
                    mm = m % 2
                    for k in range(KH):
                        nc.tensor.matmul(
                            ps,
                            lhsT=w3c[:, k, mm * P:(mm + 1) * P],
                            rhs=h[:, k, :],
                            start=(k == 0),
                            stop=(k == KH - 1),
                        )
                    nc.vector.tensor_copy(osb[:, m, :], ps)
                nc.sync.dma_start(outs[t].rearrange("(mo p) n -> p mo n", p=P), osb[:])
    nc.compile()
    return nc


def _softmax(a, axis):
    m = a.max(axis=axis, keepdims=True)
    e = np.exp(a - m)
    return e / e.sum(axis=axis, keepdims=True)


def kernel(x, meta_w, meta_b, macro_w, macro_b,
           fc1_w, fc1_b, fc2_w, fc2_b, fc3_w, fc3_b, proj_w, proj_b):
    global LAST_RESULTS, LAST_EXEC_NS

    f = np.float32
    x = np.ascontiguousarray(np.asarray(x, f))
    meta_w = np.asarray(meta_w, f); meta_b = np.asarray(meta_b, f)
    macro_w = np.asarray(macro_w, f); macro_b = np.asarray(macro_b, f)
    fc1_w = np.asarray(fc1_w, f); fc1_b = np.asarray(fc1_b, f)
    fc2_w = np.asarray(fc2_w, f); fc2_b = np.asarray(fc2_b, f)
    fc3_w = np.asarray(fc3_w, f); fc3_b = np.asarray(fc3_b, f)
    proj_w = np.asarray(proj_w, f); proj_b = np.asarray(proj_b, f)

    # ---- replicated routers (host, fp32) ----
    ml = x @ meta_w + meta_b                                   # [B, META]
    mi = np.argmax(ml, axis=1)
    macl = np.einsum("bd,mdk->bmk", x, macro_w, optimize=True) + macro_b
    sel = np.take_along_axis(macl, mi[:, None, None], axis=1)[:, 0, :]
    ma = np.argmax(sel, axis=1)
    grp = mi * MACRO + ma                                      # [B]

    # ---- aux load-balancing loss ----
    mp = _softmax(ml, 1).mean(axis=0)
    aux = f(ALPHA * META) * np.sum(mp * mp)
    mpr = _softmax(macl, 2)                                    # [B, META, MACRO]
    mask = (mi[:, None] == np.arange(META)[None, :]).astype(f)
    count = mask.sum(axis=0)
    mean_p = np.einsum("bmk,bm->mk", mpr, mask) / np.maximum(count, 1.0)[:, None]
    loss_m = f(ALPHA * MACRO) * np.sum(mean_p * mean_p, axis=1) * (count > 0)
    aux = aux + loss_m.sum()

    # ---- dispatch: gather tokens by group, pad to uniform tile T ----
    idx = [np.flatnonzero(grp == g) for g in range(G)]
    maxn = max(len(i) for i in idx)
    T = max(256, -(-maxn // P) * P)
    xtg = []
    for g in range(G):
        a = np.zeros((D, T), f)
        n = len(idx[g])
        if n:
            a[:, :n] = x[idx[g]].T
        xtg.append(a)

    nc = _NC_CACHE.get(T)
    if nc is None:
        nc = _build_nc(T)
        _NC_CACHE[T] = nc

    in_maps = []
    for c in range(NCORES):
        m = {}
        for t in range(EPC):
            e = EPC * c + t
            g = e // MICRO
            m[f"xt{t}"] = xtg[g]
            m[f"w1_{t}"] = np.ascontiguousarray(fc1_w[e])
            m[f"w2_{t}"] = np.ascontiguousarray(fc2_w[e])
            m[f"w3_{t}"] = np.ascontiguousarray(fc3_w[e])
            m[f"wp_{t}"] = np.ascontiguousarray(proj_w[e])
            m[f"bb_{t}"] = np.ascontiguousarray(np.concatenate(
                [fc1_b[e].reshape(KH, P).T,
                 (fc2_b[e] + proj_b[e]).reshape(KH, P).T], axis=1))
        in_maps.append(m)

    from concourse.bass_utils import run_bass_kernel_spmd

    trace = os.environ.get("KERNEL_TRACE", "0") == "1"
    res = run_bass_kernel_spmd(nc, in_maps, core_ids=list(range(NCORES)), trace=trace)
    LAST_RESULTS = res
    LAST_EXEC_NS = res.exec_time_ns

    # ---- combine: mean over the 4 micro experts, unpermute ----
    final = np.zeros((B, O), f)
    for g in range(G):
        n = len(idx[g])
        if n == 0:
            continue
        acc = np.zeros((O, T), f)
        for j in range(MICRO):
            e = MICRO * g + j
            c, t = divmod(e, EPC)
            acc = acc + res.results[c][f"out{t}"]
        b3m = fc3_w.dtype.type(0.25) * fc3_b[MICRO * g:MICRO * g + MICRO].sum(axis=0)
        final[idx[g]] = acc[:, :n].T * f(0.25) + b3m
    return final, np.asarray(aux, f)
